# revision 53
# baseline (speedup 1.0000x reference)
"""Order-2 CRF NLL loss kernel for Trainium2 (8 NeuronCores, Bass/Tile).

Strategy (v3)
-------------
Data-parallel over the batch, but length-aware: the mask is a valid-prefix
mask with random lengths, so roughly half of all scan steps are masked.
The host packs ONLY the unmasked steps of each sequence and assigns
sequences to (core, chain) slots by length rank, so the (runtime-built)
program's per-chain capacities adapt to the actual mask:

  - sequences sorted by #scan-steps U descending; rank i -> core i%8,
    chain i//8.  Chain c's capacity C_c = max U over its 8 sequences,
    rounded up to 16 (identity padding at the tail).
  - chains placed to balance the two partition halves: chains {0,3} on
    partitions 0-63, {1,2} on 64-127 (pairs long with short).

The CRF forward recursion runs in the exp domain: a <- Mhat^T a with
Mhat = exp(E - c0), c0 = log(64)+0.5; host precomputes Mhat in bf16
(halving DMA) and the gold score; logZ_b = log(sum a_final) + c0*(U_b+1).

The product tree is depth 3 (octs): each group of 16 positions forms
4 pair products (P2), 4 quad products (P4), 2 oct products (P8) on the
PE, and the serial scan applies one P8 per 8 steps -- ~C/8 dependent
matvecs per chain.  The transpose-free trick stores positions 0,2 mod 4
host-transposed; even quads compute their P4 operand-swapped so every
product is lhsT.T @ rhs with no device transposes.

Each (chunk, half) of packed steps is one plain 2D DMA with multi-KB
contiguous rows.
"""

import numpy as np
import ml_dtypes

import concourse.bass as bass
import concourse.tile as tile
from concourse import mybir
from concourse.bass_utils import run_bass_kernel_spmd

# ---------------------------------------------------------------- constants
B, S, L = 32, 512, 64
NCORES = 8
C0 = float(np.log(L) + 0.5)
SHIFT = 3.0  # fp8 range shift: inputs exp(E - C0 + SHIFT), P2 cast /e^{2*SHIFT}
PADV = 16.0  # pad matrices are PADV*I; 16 is exact in fp8 and close to e^SHIFT
F32 = mybir.dt.float32
BF16 = mybir.dt.bfloat16
FP8 = mybir.dt.float8e4
AF = mybir.ActivationFunctionType
BF16NP = ml_dtypes.bfloat16
FP8NP = ml_dtypes.float8_e4m3fn

# chain placement: (partition base, alpha/output column)
CHHOME = [0, 64, 64, 0]
CHACOL = [0, 0, 1, 1]
GRP = 16          # positions per product group (4 quads -> 2 octs)
CHUNK = 64        # positions per DMA chunk (4 groups)


def split_multi_waits(nc, max_waits=1):
    """This walrus build accepts at most one sync-wait per instruction;
    move extra waits onto NOPs inserted just before, same engine."""
    for fn in nc.m.functions:
        for bb in fn.blocks:
            newl = []
            for ins in bb.instructions:
                si = ins.sync_info
                if si is not None and si.on_wait and len(si.on_wait) > max_waits:
                    waits = list(si.on_wait)
                    keep = waits[:max_waits]
                    extra = waits[max_waits:]
                    for i in range(0, len(extra), max_waits):
                        nop = mybir.InstNoOp(
                            name=nc.get_next_instruction_name(),
                            ins=[],
                            outs=[],
                            sync_info=mybir.SyncInfo(
                                on_wait=extra[i : i + max_waits], on_update=[]
                            ),
                        )
                        nop.engine = ins.engine
                        newl.append(nop)
                    si.on_wait = keep
                newl.append(ins)
            bb.instructions[:] = newl


def _chunk_layout(caps):
    """Static per-chunk layout shared by program and host packing.

    Returns a list of chunk dicts:
      k, lo (global position), npos, per-half: active chain list,
      region offset into that half's packed host array, region cols.
    """
    # graduated chunk sizes: small first chunks so the PE starts early
    los = [0, 16, 64]
    while los[-1] < max(caps):
        los.append(los[-1] + CHUNK)
    los = [x for x in los if x < max(caps)]
    chunks = []
    off = {0: 0, 64: 0}
    for k, lo in enumerate(los):
        nxt = los[k + 1] if k + 1 < len(los) else max(caps)
        npos = nxt - lo
        halves = {}
        for h in (0, 64):
            act = [c for c in range(4) if CHHOME[c] == h and caps[c] > lo]
            # all active chains cover the full chunk except possibly the
            # last positions; npos per half:
            nph = 0
            if act:
                nph = min(npos, max(caps[c] for c in act) - lo)
            halves[h] = dict(act=act, off=off[h], npos=nph)
            off[h] += nph * 64 * len(act)
        chunks.append(dict(k=k, lo=lo, halves=halves))
    return chunks, off[0], off[64]


def build_nc(caps, resets):
    """caps: tuple of 4 per-slot capacities (multiples of GRP).
    resets: per slot, tuple of (start_oct, end_oct, piece_col)."""
    chunks, totA, totB = _chunk_layout(caps)
    p8scale = float(np.exp(-8.0 * SHIFT))
    startmap = [dict() for _ in range(4)]
    endmap = [dict() for _ in range(4)]
    for s in range(4):
        for (o0, oend, pcol) in resets[s]:
            startmap[s][o0] = pcol
            endmap[s][oend] = pcol

    nc = bass.Bass()
    emA = nc.dram_tensor("emA", [64, totA], FP8, kind="ExternalInput")
    emB = nc.dram_tensor("emB", [64, totB], FP8, kind="ExternalInput")
    a0_d = nc.dram_tensor("a0", [128, 4], BF16, kind="ExternalInput")
    out_d = nc.dram_tensor("out", [128, 4], F32, kind="ExternalOutput")

    def em_ap(h, offset, ap):
        t = (emA if h == 0 else emB)[:, :].tensor
        return bass.AP(tensor=t, offset=offset, ap=ap)

    with tile.TileContext(nc) as tc:
        with (
            tc.tile_pool(name="expp", bufs=3) as expp,
            tc.tile_pool(name="p2sb", bufs=2) as p2sbp,
            tc.tile_pool(name="p4sb", bufs=2) as p4sbp,
            tc.tile_pool(name="p8sb", bufs=3) as p8sbp,
            tc.tile_pool(name="alpha", bufs=4) as alphap,
            tc.tile_pool(name="small", bufs=1) as small,
            tc.tile_pool(name="pp2", bufs=2, space="PSUM") as pp2p,
            tc.tile_pool(name="pp4", bufs=2, space="PSUM") as pp4p,
            tc.tile_pool(name="pp8", bufs=2, space="PSUM") as pp8p,
        ):
            # ---------------- piece init vectors (host-prepared, exp domain)
            a0sb = small.tile([128, 4], BF16)
            nc.sync.dma_start(out=a0sb[:, :], in_=a0_d[:, :])
            # per-piece final alphas are snapshotted here as pieces finish
            osb = small.tile([128, 4], F32)
            # alpha layout [128, 2]: slot c at (CHHOME[c], CHACOL[c]);
            # every slot's oct 0 is a piece start, so this is never read
            # before the first cast writes it
            alpha = small.tile([128, 2], BF16)

            # ---------------- main pipeline over chunks
            for ch in chunks:
                lo = ch["lo"]
                hv = ch["halves"]
                ncols = {h: hv[h]["npos"] * 64 * len(hv[h]["act"]) for h in (0, 64)}
                et = expp.tile([128, max(ncols[0], ncols[64])], FP8, tag="exp")
                # issue bulk DMAs from the otherwise-idle gpsimd queue so
                # the sync queue's serial dma_start cost is off the ramp
                for h in (0, 64):
                    if ncols[h]:
                        tot = totA if h == 0 else totB
                        nc.gpsimd.dma_start(
                            out=et[h : h + 64, 0 : ncols[h]],
                            in_=em_ap(
                                h, hv[h]["off"], [[tot, 64], [1, ncols[h]]]
                            ),
                        )

                def esl(c, p):
                    # position p (global), chain c: slice of et
                    h = CHHOME[c]
                    a = hv[h]["act"]
                    off = (p - lo) * 64 * len(a) + 64 * a.index(c)
                    return et[h : h + 64, off : off + 64]

                # groups of GRP=16 positions
                glo = lo
                while glo < lo + max(
                    (hv[h]["npos"] for h in (0, 64) if hv[h]["act"]), default=0
                ):
                    gact = [c for c in range(4) if caps[c] > glo]
                    h0l = [c for c in gact if CHHOME[c] == 0]
                    h1l = [c for c in gact if CHHOME[c] == 64]
                    nh = max(len(h0l), len(h1l))
                    # emission order alternates partition halves so each
                    # quadrant's LDWEIGHTS prefetches during the other
                    # half's matmul
                    ordc = []
                    for i in range(nh):
                        if i < len(h0l):
                            ordc.append(h0l[i])
                        if i < len(h1l):
                            ordc.append(h1l[i])

                    def cix(c):
                        h = CHHOME[c]
                        return (h0l if h == 0 else h1l).index(c)

                    # P2 level: 8 pair products per chain (4 quads x 2)
                    pp2 = pp2p.tile([128, 1024], F32, tag="pp2")
                    for q in range(4):
                        p0 = glo + 4 * q
                        for half in range(2):
                            for c in ordc:
                                h = CHHOME[c]
                                cb = cix(c) * 512 + q * 128 + 64 * half
                                lhsT = esl(c, p0 + 1 + half)
                                rhs = esl(c, p0 + 3 * half)
                                nc.tensor.matmul(
                                    out=pp2[h : h + 64, cb : cb + 64],
                                    lhsT=lhsT,
                                    rhs=rhs,
                                    start=True,
                                    stop=True,
                                    tile_position=(h, h),
                                )
                    p2sb = p2sbp.tile([128, 1024], BF16, tag="p2sb")
                    uc = 512 * nh
                    nc.scalar.activation(
                        out=p2sb[:, 0 : uc // 2],
                        in_=pp2[:, 0 : uc // 2],
                        func=AF.Copy,
                    )
                    nc.vector.tensor_copy(
                        out=p2sb[:, uc // 2 : uc], in_=pp2[:, uc // 2 : uc]
                    )

                    # P4 level: 4 per chain; even quads operand-swapped so
                    # their P4 comes out transposed-stored
                    pp4 = pp4p.tile([128, 512], F32, tag="pp4")
                    for q in range(4):
                        for c in ordc:
                            h = CHHOME[c]
                            cb = cix(c) * 512 + q * 128
                            ob = cix(c) * 256 + q * 64
                            a_sl = p2sb[h : h + 64, cb : cb + 64]
                            b_sl = p2sb[h : h + 64, cb + 64 : cb + 128]
                            lhsT, rhs = (b_sl, a_sl) if q % 2 == 0 else (a_sl, b_sl)
                            nc.tensor.matmul(
                                out=pp4[h : h + 64, ob : ob + 64],
                                lhsT=lhsT,
                                rhs=rhs,
                                start=True,
                                stop=True,
                                tile_position=(h, h),
                            )
                    p4sb = p4sbp.tile([128, 512], BF16, tag="p4sb")
                    uc = 256 * nh
                    nc.scalar.activation(
                        out=p4sb[:, 0 : uc // 2], in_=pp4[:, 0 : uc // 2], func=AF.Copy
                    )
                    nc.vector.tensor_copy(
                        out=p4sb[:, uc // 2 : uc], in_=pp4[:, uc // 2 : uc]
                    )

                    # P8 level (2 per chain) + scan outputs share one tile
                    pp8 = pp8p.tile([128, 264], F32, tag="pp8")
                    for o in range(2):
                        for c in ordc:
                            h = CHHOME[c]
                            ci = cix(c)
                            ob4 = ci * 256 + o * 128
                            nc.tensor.matmul(
                                out=pp8[h : h + 64, ci * 128 + o * 64 :][:, 0:64],
                                lhsT=p4sb[h : h + 64, ob4 : ob4 + 64],
                                rhs=p4sb[h : h + 64, ob4 + 64 : ob4 + 128],
                                start=True,
                                stop=True,
                                tile_position=(h, h),
                            )
                    p8sb = p8sbp.tile([128, 256], BF16, tag="p8sb")
                    uc = 128 * nh
                    # the whole fp8 shift is paid once here: /e^{8*SHIFT}
                    nc.scalar.activation(
                        out=p8sb[:, 0 : uc // 2],
                        in_=pp8[:, 0 : uc // 2],
                        func=AF.Copy,
                        scale=p8scale,
                    )
                    nc.vector.tensor_scalar_mul(
                        out=p8sb[:, uc // 2 : uc],
                        in0=pp8[:, uc // 2 : uc],
                        scalar1=p8scale,
                    )

                    # scan: apply the two P8s in order; one batched
                    # [128,2] cast per oct covers all active chains.
                    # Piece starts read from a0sb; piece ends snapshot
                    # their final alpha to osb.
                    for o in range(2):
                        oct_i = glo // 8 + o
                        sc = 256 + 2 * o
                        for c in ordc:
                            h = CHHOME[c]
                            ci = cix(c)
                            if oct_i in startmap[c]:
                                pcol = startmap[c][oct_i]
                                rhs = a0sb[h : h + 64, pcol : pcol + 1]
                            else:
                                rhs = alpha[h : h + 64, CHACOL[c] : CHACOL[c] + 1]
                            nc.tensor.matmul(
                                out=pp8[h : h + 64, sc + CHACOL[c] :][:, 0:1],
                                lhsT=p8sb[h : h + 64, ci * 128 + o * 64 :][:, 0:64],
                                rhs=rhs,
                                start=True,
                                stop=True,
                                tile_position=(h, h),
                            )
                        at = alphap.tile([128, 2], BF16, tag="alpha")
                        nc.vector.tensor_copy(
                            out=at[:, :], in_=pp8[:, sc : sc + 2]
                        )
                        alpha = at
                        for c in gact:
                            if oct_i in endmap[c]:
                                h = CHHOME[c]
                                pcol = endmap[c][oct_i]
                                nc.vector.tensor_copy(
                                    out=osb[h : h + 64, pcol : pcol + 1],
                                    in_=alpha[h : h + 64, CHACOL[c] : CHACOL[c] + 1],
                                )
                    glo += GRP

            # ---------------- finale: ship final alphas (fp32) to host
            nc.sync.dma_start(out=out_d[:, :], in_=osb[:, :])

    split_multi_waits(nc)
    return nc


_NC_CACHE = {}


def _get_nc(rkey):
    if rkey not in _NC_CACHE:
        caps, resets = rkey
        _NC_CACHE[rkey] = build_nc(caps, resets)
    return _NC_CACHE[rkey]


def _r16(x):
    return int(-(-int(x) // GRP) * GRP)


def _layout(U, seq_of):
    """Choose the slot/piece layout.  A piece is (band, kind, cell) with
    kind in {w, f, bw}; slot caps are the cell sums.  Layout A gives each
    band its own slot; layout B splits band 0 fwd/bwd to shorten the
    longest slot (helps when band 0 is much longer than the rest)."""
    bmax = [max(int(U[seq_of[j, c]]) for j in range(NCORES)) for c in range(4)]
    la = [[(c, "w", _r16(bmax[c]))] for c in range(4)]
    cands = [la]
    F = _r16(bmax[0] / 2)
    bw = _r16(max(bmax[0] - F, 0))
    if bw > 0:
        lb = [
            [(0, "f", F)],
            [(0, "bw", bw)],
            [(1, "w", _r16(bmax[1]))],
            [(2, "w", _r16(bmax[2])), (3, "w", _r16(bmax[3]))],
        ]
        cands.append(lb)
    return min(
        cands,
        key=lambda l: (sum(sum(p[2] for p in s) for s in l),
                       max(sum(p[2] for p in s) for s in l)),
    )


def prepare_inputs(emits, targets, mask):
    """Host-side prep: seq assignment, capacities, packed per-core arrays."""
    emits = np.asarray(emits, dtype=np.float32)
    maskb = np.asarray(mask).astype(bool)
    U = maskb[:, 1:].sum(axis=1).astype(np.int64)  # scan steps per seq

    # rank i (by U desc) -> core i%8, band i//8
    order = np.argsort(-U, kind="stable")
    seq_of = np.empty((NCORES, 4), dtype=np.int64)
    for i, b in enumerate(order):
        seq_of[i % NCORES, i // NCORES] = b
    layout = _layout(U, seq_of)
    caps = tuple(sum(p[2] for p in s) for s in layout)
    # resets: per slot, (start_oct, end_oct, piece_col); piece_col is a
    # globally unique column per (half), assigned per slot then piece
    resets = []
    pmeta = []  # (slot, band, kind, cell, pcol, start_pos)
    for s, pieces in enumerate(layout):
        pos = 0
        rs = []
        for k, (band, kind, cell) in enumerate(pieces):
            pcol = CHACOL[s] * 2 + k  # <=2 pieces per slot
            rs.append((pos // 8, (pos + cell) // 8 - 1, pcol))
            pmeta.append((s, band, kind, cell, pcol, pos))
            pos += cell
        resets.append(tuple(rs))
    rkey = (caps, tuple(resets))

    E4 = emits.reshape(B, S, L, L)
    # fp8 matrices: exp(E - C0 + SHIFT); the combined shift is divided
    # back out at the P8 cast (/e^{8*SHIFT})
    X8 = np.exp(E4 - (C0 - SHIFT)).astype(FP8NP)                # [B,S,L,L]
    a0f = np.exp(E4[:, 0, 0, :] - C0).astype(BF16NP)            # [B, L]
    # pads are PADV*I (PADV exact in fp8, ~e^SHIFT so pads never underflow
    # bf16 alpha); the host adds (SHIFT - log PADV) per pad to logZ
    iden = (PADV * np.eye(L, dtype=np.float32)).astype(FP8NP)
    chunks, totA, totB = _chunk_layout(caps)

    in_maps = []
    for j in range(NCORES):
        emAa = np.empty((64, totA), dtype=FP8NP)
        emBa = np.empty((64, totB), dtype=FP8NP)
        a0 = np.zeros((128, 4), dtype=BF16NP)
        for c in range(4):
            h = CHHOME[c]
            G = np.empty((caps[c], L, L), dtype=FP8NP)
            G[:] = iden
            for (s, band, kind, cell, pcol, pos) in pmeta:
                if s != c:
                    continue
                b = seq_of[j, band]
                u = int(U[b])
                if kind == "w":
                    G[pos : pos + u] = X8[b, 1 : u + 1]
                    a0[h : h + 64, pcol] = a0f[b]
                elif kind == "f":
                    n = min(u, cell)
                    G[pos : pos + n] = X8[b, 1 : n + 1]
                    a0[h : h + 64, pcol] = a0f[b]
                else:  # bw: steps F+1..u reversed, transposed
                    n = max(u - (cell_f := next(
                        cc for (s2, b2, k2, cc, _, _) in pmeta
                        if b2 == band and k2 == "f"
                    )), 0)
                    if n > 0:
                        blk = X8[b, u - n + 1 : u + 1][::-1]
                        G[pos : pos + n] = np.ascontiguousarray(
                            blk.swapaxes(-1, -2)
                        )
                    a0[h : h + 64, pcol] = 1.0
            G[0::2] = np.ascontiguousarray(G[0::2].swapaxes(-1, -2))
            arr = emAa if h == 0 else emBa
            for ch in chunks:
                hb = ch["halves"][h]
                act = hb["act"]
                if c not in act:
                    continue
                lo = ch["lo"]
                npos = min(hb["npos"], caps[c] - lo)
                view = arr[:, hb["off"] : hb["off"] + hb["npos"] * 64 * len(act)]
                view = view.reshape(64, hb["npos"], len(act), 64)
                view[:, 0:npos, act.index(c), :] = G[lo : lo + npos].transpose(
                    1, 0, 2
                )
        in_maps.append({"emA": emAa, "emB": emBa, "a0": a0})
    return in_maps, maskb, rkey, pmeta, seq_of, U


def host_score(emits, targets, maskb):
    tg = np.asarray(targets).astype(np.int64)
    idx = tg[:, :-1] * L + tg[:, 1:]                 # [B, S]
    em = np.asarray(emits, dtype=np.float64).reshape(B, S, L * L)
    gold = np.take_along_axis(em, idx[:, :, None], axis=-1)[..., 0]
    return float(np.where(maskb, gold, 0.0).sum())


def assemble_loss(results, maskb, score, pmeta, seq_of, U):
    # each PADV*I pad step nets alpha *= PADV/e^SHIFT through the P8
    # cast's shift; compensate exactly per piece
    padc = SHIFT - np.log(PADV)
    logZ = 0.0
    for j in range(NCORES):
        o = np.asarray(results[j]["out"], dtype=np.float64)
        vec = {}
        for (s, band, kind, cell, pcol, pos) in pmeta:
            h = CHHOME[s]
            vec[(band, kind)] = (o[h : h + 64, pcol], cell)
        for band in range(4):
            b = seq_of[j, band]
            u = int(U[b])
            if (band, "w") in vec:
                v, cell = vec[(band, "w")]
                z = v.sum()
                npad = cell - u
            else:
                vf, cf = vec[(band, "f")]
                vb, cb = vec[(band, "bw")]
                nf = min(u, cf)
                if u <= cf:
                    z = vf.sum()  # whole seq fit in the fwd piece
                    npad = cf - u  # bw piece is all pads & unused
                else:
                    z = vf @ vb
                    npad = (cf - nf) + (cb - (u - nf))
            logZ += np.log(z) + C0 * (u + 1) + padc * npad
    total_token = float(maskb.sum())
    return np.float32((logZ - score) / total_token)


def kernel(emits, targets, mask, _trace=False):
    in_maps, maskb, rkey, pmeta, seq_of, U = prepare_inputs(emits, targets, mask)
    score = host_score(emits, targets, maskb)
    nc = _get_nc(rkey)
    res = run_bass_kernel_spmd(nc, in_maps, core_ids=list(range(NCORES)), trace=_trace)
    loss = assemble_loss(res.results, maskb, score, pmeta, seq_of, U)
    if _trace:
        return loss, res
    return loss


# revision 54
# speedup vs baseline: 1.0799x; 1.0799x over previous
"""Order-2 CRF NLL loss kernel for Trainium2 (8 NeuronCores, Bass/Tile).

Strategy (v3)
-------------
Data-parallel over the batch, but length-aware: the mask is a valid-prefix
mask with random lengths, so roughly half of all scan steps are masked.
The host packs ONLY the unmasked steps of each sequence and assigns
sequences to (core, chain) slots by length rank, so the (runtime-built)
program's per-chain capacities adapt to the actual mask:

  - sequences sorted by #scan-steps U descending; rank i -> core i%8,
    chain i//8.  Chain c's capacity C_c = max U over its 8 sequences,
    rounded up to 16 (identity padding at the tail).
  - chains placed to balance the two partition halves: chains {0,3} on
    partitions 0-63, {1,2} on 64-127 (pairs long with short).

The CRF forward recursion runs in the exp domain: a <- Mhat^T a with
Mhat = exp(E - c0), c0 = log(64)+0.5; host precomputes Mhat in bf16
(halving DMA) and the gold score; logZ_b = log(sum a_final) + c0*(U_b+1).

The product tree is depth 3 (octs): each group of 16 positions forms
4 pair products (P2), 4 quad products (P4), 2 oct products (P8) on the
PE, and the serial scan applies one P8 per 8 steps -- ~C/8 dependent
matvecs per chain.  The transpose-free trick stores positions 0,2 mod 4
host-transposed; even quads compute their P4 operand-swapped so every
product is lhsT.T @ rhs with no device transposes.

Each (chunk, half) of packed steps is one plain 2D DMA with multi-KB
contiguous rows.
"""

import numpy as np
import ml_dtypes

import concourse.bass as bass
import concourse.tile as tile
from concourse import mybir
from concourse.bass_utils import run_bass_kernel_spmd

# ---------------------------------------------------------------- constants
B, S, L = 32, 512, 64
NCORES = 8
C0 = float(np.log(L) + 0.5)
SHIFT = 3.0  # fp8 range shift: inputs exp(E - C0 + SHIFT), P2 cast /e^{2*SHIFT}
PADV = 16.0  # pad matrices are PADV*I; 16 is exact in fp8 and close to e^SHIFT
F32 = mybir.dt.float32
BF16 = mybir.dt.bfloat16
FP8 = mybir.dt.float8e4
AF = mybir.ActivationFunctionType
BF16NP = ml_dtypes.bfloat16
FP8NP = ml_dtypes.float8_e4m3fn

# chain placement: (partition base, alpha/output column)
CHHOME = [0, 64, 64, 0]
CHACOL = [0, 0, 1, 1]
GRP = 16          # positions per product group (4 quads -> 2 octs)
CHUNK = 64        # positions per DMA chunk (4 groups)


def split_multi_waits(nc, max_waits=1):
    """This walrus build accepts at most one sync-wait per instruction;
    move extra waits onto NOPs inserted just before, same engine."""
    for fn in nc.m.functions:
        for bb in fn.blocks:
            newl = []
            for ins in bb.instructions:
                si = ins.sync_info
                if si is not None and si.on_wait and len(si.on_wait) > max_waits:
                    waits = list(si.on_wait)
                    keep = waits[:max_waits]
                    extra = waits[max_waits:]
                    for i in range(0, len(extra), max_waits):
                        nop = mybir.InstNoOp(
                            name=nc.get_next_instruction_name(),
                            ins=[],
                            outs=[],
                            sync_info=mybir.SyncInfo(
                                on_wait=extra[i : i + max_waits], on_update=[]
                            ),
                        )
                        nop.engine = ins.engine
                        newl.append(nop)
                    si.on_wait = keep
                newl.append(ins)
            bb.instructions[:] = newl


def _chunk_layout(caps):
    """Static per-chunk layout shared by program and host packing.

    Returns a list of chunk dicts:
      k, lo (global position), npos, per-half: active chain list,
      region offset into that half's packed host array, region cols.
    """
    # graduated chunk sizes: small first chunks so the PE starts early
    los = [0, 16, 64]
    while los[-1] < max(caps):
        los.append(los[-1] + CHUNK)
    los = [x for x in los if x < max(caps)]
    chunks = []
    off = {0: 0, 64: 0}
    for k, lo in enumerate(los):
        nxt = los[k + 1] if k + 1 < len(los) else max(caps)
        npos = nxt - lo
        halves = {}
        for h in (0, 64):
            act = [c for c in range(4) if CHHOME[c] == h and caps[c] > lo]
            # all active chains cover the full chunk except possibly the
            # last positions; npos per half:
            nph = 0
            if act:
                nph = min(npos, max(caps[c] for c in act) - lo)
            halves[h] = dict(act=act, off=off[h], npos=nph)
            off[h] += nph * 64 * len(act)
        chunks.append(dict(k=k, lo=lo, halves=halves))
    return chunks, off[0], off[64]


def build_nc(caps):
    """caps: tuple of 4 per-chain capacities (multiples of GRP)."""
    chunks, totA, totB = _chunk_layout(caps)
    p8scale = float(np.exp(-8.0 * SHIFT))

    nc = bass.Bass()
    emA = nc.dram_tensor("emA", [64, totA], FP8, kind="ExternalInput")
    emB = nc.dram_tensor("emB", [64, totB], FP8, kind="ExternalInput")
    a0_d = nc.dram_tensor("a0", [128, 2], BF16, kind="ExternalInput")
    out_d = nc.dram_tensor("out", [128, 2], F32, kind="ExternalOutput")

    def em_ap(h, offset, ap):
        t = (emA if h == 0 else emB)[:, :].tensor
        return bass.AP(tensor=t, offset=offset, ap=ap)

    with tile.TileContext(nc) as tc:
        with (
            tc.tile_pool(name="expp", bufs=4) as expp,
            tc.tile_pool(name="p2sb", bufs=2) as p2sbp,
            tc.tile_pool(name="p4sb", bufs=2) as p4sbp,
            tc.tile_pool(name="p8sb", bufs=4) as p8sbp,
            tc.tile_pool(name="alpha", bufs=6) as alphap,
            tc.tile_pool(name="small", bufs=1) as small,
            tc.tile_pool(name="pp2", bufs=2, space="PSUM") as pp2p,
            tc.tile_pool(name="pp4", bufs=2, space="PSUM") as pp4p,
            tc.tile_pool(name="pp8", bufs=2, space="PSUM") as pp8p,
        ):
            # ---------------- init: alpha0 (host-prepared, exp domain)
            # alpha layout [128, 2]: chain c at (CHHOME[c], CHACOL[c])
            alpha = small.tile([128, 2], BF16)
            nc.sync.dma_start(out=alpha[:, :], in_=a0_d[:, :])
            # final alphas are snapshotted here as each chain finishes
            # (later batched scan casts clobber finished chains' columns)
            osb = small.tile([128, 2], F32)

            # ---------------- main pipeline over chunks
            for ch in chunks:
                lo = ch["lo"]
                hv = ch["halves"]
                ncols = {h: hv[h]["npos"] * 64 * len(hv[h]["act"]) for h in (0, 64)}
                et = expp.tile([128, max(ncols[0], ncols[64])], FP8, tag="exp")
                # issue bulk DMAs from the otherwise-idle gpsimd queue so
                # the sync queue's serial dma_start cost is off the ramp
                for h in (0, 64):
                    if ncols[h]:
                        tot = totA if h == 0 else totB
                        nc.gpsimd.dma_start(
                            out=et[h : h + 64, 0 : ncols[h]],
                            in_=em_ap(
                                h, hv[h]["off"], [[tot, 64], [1, ncols[h]]]
                            ),
                        )

                def esl(c, p):
                    # position p (global), chain c: slice of et
                    h = CHHOME[c]
                    a = hv[h]["act"]
                    off = (p - lo) * 64 * len(a) + 64 * a.index(c)
                    return et[h : h + 64, off : off + 64]

                # groups of GRP=16 positions
                glo = lo
                while glo < lo + max(
                    (hv[h]["npos"] for h in (0, 64) if hv[h]["act"]), default=0
                ):
                    gact = [c for c in range(4) if caps[c] > glo]
                    h0l = [c for c in gact if CHHOME[c] == 0]
                    h1l = [c for c in gact if CHHOME[c] == 64]
                    nh = max(len(h0l), len(h1l))
                    # emission order alternates partition halves so each
                    # quadrant's LDWEIGHTS prefetches during the other
                    # half's matmul
                    ordc = []
                    for i in range(nh):
                        if i < len(h0l):
                            ordc.append(h0l[i])
                        if i < len(h1l):
                            ordc.append(h1l[i])

                    def cix(c):
                        h = CHHOME[c]
                        return (h0l if h == 0 else h1l).index(c)

                    # P2 level: 8 pair products per chain (4 quads x 2)
                    pp2 = pp2p.tile([128, 1024], F32, tag="pp2")
                    for q in range(4):
                        p0 = glo + 4 * q
                        for half in range(2):
                            for c in ordc:
                                h = CHHOME[c]
                                cb = cix(c) * 512 + q * 128 + 64 * half
                                lhsT = esl(c, p0 + 1 + half)
                                rhs = esl(c, p0 + 3 * half)
                                nc.tensor.matmul(
                                    out=pp2[h : h + 64, cb : cb + 64],
                                    lhsT=lhsT,
                                    rhs=rhs,
                                    start=True,
                                    stop=True,
                                    tile_position=(h, h),
                                )
                    p2sb = p2sbp.tile([128, 1024], BF16, tag="p2sb")
                    uc = 512 * nh
                    nc.scalar.activation(
                        out=p2sb[:, 0 : uc // 2],
                        in_=pp2[:, 0 : uc // 2],
                        func=AF.Copy,
                    )
                    nc.vector.tensor_copy(
                        out=p2sb[:, uc // 2 : uc], in_=pp2[:, uc // 2 : uc]
                    )

                    # P4 level: 4 per chain; even quads operand-swapped so
                    # their P4 comes out transposed-stored
                    pp4 = pp4p.tile([128, 512], F32, tag="pp4")
                    for q in range(4):
                        for c in ordc:
                            h = CHHOME[c]
                            cb = cix(c) * 512 + q * 128
                            ob = cix(c) * 256 + q * 64
                            a_sl = p2sb[h : h + 64, cb : cb + 64]
                            b_sl = p2sb[h : h + 64, cb + 64 : cb + 128]
                            lhsT, rhs = (b_sl, a_sl) if q % 2 == 0 else (a_sl, b_sl)
                            nc.tensor.matmul(
                                out=pp4[h : h + 64, ob : ob + 64],
                                lhsT=lhsT,
                                rhs=rhs,
                                start=True,
                                stop=True,
                                tile_position=(h, h),
                            )
                    p4sb = p4sbp.tile([128, 512], BF16, tag="p4sb")
                    uc = 256 * nh
                    nc.scalar.activation(
                        out=p4sb[:, 0 : uc // 2], in_=pp4[:, 0 : uc // 2], func=AF.Copy
                    )
                    nc.vector.tensor_copy(
                        out=p4sb[:, uc // 2 : uc], in_=pp4[:, uc // 2 : uc]
                    )

                    # P8 level (2 per chain) + scan outputs share one tile
                    pp8 = pp8p.tile([128, 264], F32, tag="pp8")
                    for o in range(2):
                        for c in ordc:
                            h = CHHOME[c]
                            ci = cix(c)
                            ob4 = ci * 256 + o * 128
                            nc.tensor.matmul(
                                out=pp8[h : h + 64, ci * 128 + o * 64 :][:, 0:64],
                                lhsT=p4sb[h : h + 64, ob4 : ob4 + 64],
                                rhs=p4sb[h : h + 64, ob4 + 64 : ob4 + 128],
                                start=True,
                                stop=True,
                                tile_position=(h, h),
                            )
                    p8sb = p8sbp.tile([128, 256], BF16, tag="p8sb")
                    uc = 128 * nh
                    # the whole fp8 shift is paid once here: /e^{8*SHIFT}
                    nc.scalar.activation(
                        out=p8sb[:, 0 : uc // 2],
                        in_=pp8[:, 0 : uc // 2],
                        func=AF.Copy,
                        scale=p8scale,
                    )
                    nc.vector.tensor_scalar_mul(
                        out=p8sb[:, uc // 2 : uc],
                        in0=pp8[:, uc // 2 : uc],
                        scalar1=p8scale,
                    )

                    # scan: apply the two P8s in order; one batched
                    # [128,2] cast per oct covers all active chains
                    for o in range(2):
                        sc = 256 + 2 * o
                        for c in ordc:
                            h = CHHOME[c]
                            ci = cix(c)
                            nc.tensor.matmul(
                                out=pp8[h : h + 64, sc + CHACOL[c] :][:, 0:1],
                                lhsT=p8sb[h : h + 64, ci * 128 + o * 64 :][:, 0:64],
                                rhs=alpha[h : h + 64, CHACOL[c] : CHACOL[c] + 1],
                                start=True,
                                stop=True,
                                tile_position=(h, h),
                            )
                        at = alphap.tile([128, 2], BF16, tag="alpha")
                        nc.vector.tensor_copy(
                            out=at[:, :], in_=pp8[:, sc : sc + 2]
                        )
                        alpha = at
                    for c in gact:
                        if caps[c] == glo + GRP:
                            h = CHHOME[c]
                            nc.vector.tensor_copy(
                                out=osb[h : h + 64, CHACOL[c] : CHACOL[c] + 1],
                                in_=alpha[h : h + 64, CHACOL[c] : CHACOL[c] + 1],
                            )
                    glo += GRP

            # ---------------- finale: ship final alphas (fp32) to host
            nc.sync.dma_start(out=out_d[:, :], in_=osb[:, :])

    split_multi_waits(nc)
    return nc


_NC_CACHE = {}


def _get_nc(caps):
    if caps not in _NC_CACHE:
        _NC_CACHE[caps] = build_nc(caps)
    return _NC_CACHE[caps]


def prepare_inputs(emits, targets, mask):
    """Host-side prep: seq assignment, capacities, packed per-core arrays."""
    emits = np.asarray(emits, dtype=np.float32)
    maskb = np.asarray(mask).astype(bool)
    U = maskb[:, 1:].sum(axis=1).astype(np.int64)  # scan steps per seq

    # rank i (by U desc) -> core i%8, chain i//8
    order = np.argsort(-U, kind="stable")
    seq_of = np.empty((NCORES, 4), dtype=np.int64)
    for i, b in enumerate(order):
        seq_of[i % NCORES, i // NCORES] = b
    caps = tuple(
        int(-(-max(int(U[seq_of[j, c]]) for j in range(NCORES)) // GRP) * GRP)
        for c in range(4)
    )

    E4 = emits.reshape(B, S, L, L)
    # fp8 matrices: exp(E - C0 + SHIFT); the P2 cast divides by e^{2*SHIFT}
    X8 = np.exp(E4 - (C0 - SHIFT)).astype(FP8NP)                # [B,S,L,L]
    a0f = np.exp(E4[:, 0, 0, :] - C0).astype(BF16NP)            # [B, L]
    # pads are PADV*I (PADV exact in fp8, ~e^SHIFT so pads never underflow
    # bf16 alpha); the host adds (SHIFT - log PADV) per pad to logZ
    iden = (PADV * np.eye(L, dtype=np.float32)).astype(FP8NP)
    chunks, totA, totB = _chunk_layout(caps)

    in_maps = []
    for j in range(NCORES):
        emAa = np.empty((64, totA), dtype=FP8NP)
        emBa = np.empty((64, totB), dtype=FP8NP)
        a0 = np.zeros((128, 2), dtype=BF16NP)
        for c in range(4):
            b = seq_of[j, c]
            u = int(U[b])
            h = CHHOME[c]
            # chain matrices by position: steps 1..u, identity pad to cap
            G = np.empty((caps[c], L, L), dtype=FP8NP)
            G[:u] = X8[b, 1 : u + 1]
            G[u:] = iden
            G[0::2] = np.ascontiguousarray(G[0::2].swapaxes(-1, -2))
            arr = emAa if h == 0 else emBa
            for ch in chunks:
                hb = ch["halves"][h]
                act = hb["act"]
                if c not in act:
                    continue
                lo = ch["lo"]
                npos = min(hb["npos"], caps[c] - lo)
                view = arr[:, hb["off"] : hb["off"] + hb["npos"] * 64 * len(act)]
                view = view.reshape(64, hb["npos"], len(act), 64)
                view[:, 0:npos, act.index(c), :] = G[lo : lo + npos].transpose(
                    1, 0, 2
                )
            a0[h : h + 64, CHACOL[c]] = a0f[b]
        in_maps.append({"emA": emAa, "emB": emBa, "a0": a0})
    return in_maps, maskb, caps, seq_of, U


def host_score(emits, targets, maskb):
    tg = np.asarray(targets).astype(np.int64)
    idx = tg[:, :-1] * L + tg[:, 1:]                 # [B, S]
    em = np.asarray(emits, dtype=np.float64).reshape(B, S, L * L)
    gold = np.take_along_axis(em, idx[:, :, None], axis=-1)[..., 0]
    return float(np.where(maskb, gold, 0.0).sum())


def assemble_loss(results, maskb, score, seq_of, U, caps):
    logZ = 0.0
    for j in range(NCORES):
        o = np.asarray(results[j]["out"], dtype=np.float64)
        for c in range(4):
            b = seq_of[j, c]
            h = CHHOME[c]
            s = o[h : h + 64, CHACOL[c]].sum()
            # each PADV*I pad step nets alpha *= PADV/e^SHIFT through the
            # P2 cast; compensate exactly
            npad = caps[c] - int(U[b])
            logZ += (
                np.log(s)
                + C0 * (int(U[b]) + 1)
                + (SHIFT - np.log(PADV)) * npad
            )
    total_token = float(maskb.sum())
    return np.float32((logZ - score) / total_token)


def kernel(emits, targets, mask, _trace=False):
    in_maps, maskb, caps, seq_of, U = prepare_inputs(emits, targets, mask)
    score = host_score(emits, targets, maskb)
    nc = _get_nc(caps)
    res = run_bass_kernel_spmd(nc, in_maps, core_ids=list(range(NCORES)), trace=_trace)
    loss = assemble_loss(res.results, maskb, score, seq_of, U, caps)
    if _trace:
        return loss, res
    return loss


# revision 55
# speedup vs baseline: 1.1067x; 1.0248x over previous
"""Order-2 CRF NLL loss kernel for Trainium2 (8 NeuronCores, Bass/Tile).

Strategy (v3)
-------------
Data-parallel over the batch, but length-aware: the mask is a valid-prefix
mask with random lengths, so roughly half of all scan steps are masked.
The host packs ONLY the unmasked steps of each sequence and assigns
sequences to (core, chain) slots by length rank, so the (runtime-built)
program's per-chain capacities adapt to the actual mask:

  - sequences sorted by #scan-steps U descending; rank i -> core i%8,
    chain i//8.  Chain c's capacity C_c = max U over its 8 sequences,
    rounded up to 16 (identity padding at the tail).
  - chains placed to balance the two partition halves: chains {0,3} on
    partitions 0-63, {1,2} on 64-127 (pairs long with short).

The CRF forward recursion runs in the exp domain: a <- Mhat^T a with
Mhat = exp(E - c0), c0 = log(64)+0.5; host precomputes Mhat in bf16
(halving DMA) and the gold score; logZ_b = log(sum a_final) + c0*(U_b+1).

The product tree is depth 3 (octs): each group of 16 positions forms
4 pair products (P2), 4 quad products (P4), 2 oct products (P8) on the
PE, and the serial scan applies one P8 per 8 steps -- ~C/8 dependent
matvecs per chain.  The transpose-free trick stores positions 0,2 mod 4
host-transposed; even quads compute their P4 operand-swapped so every
product is lhsT.T @ rhs with no device transposes.

Each (chunk, half) of packed steps is one plain 2D DMA with multi-KB
contiguous rows.
"""

import numpy as np
import ml_dtypes

import concourse.bass as bass
import concourse.tile as tile
from concourse import mybir
from concourse.bass_utils import run_bass_kernel_spmd

# ---------------------------------------------------------------- constants
B, S, L = 32, 512, 64
NCORES = 8
C0 = float(np.log(L) + 0.5)
SHIFT = 3.0  # fp8 range shift: inputs exp(E - C0 + SHIFT), P2 cast /e^{2*SHIFT}
PADV = 16.0  # pad matrices are PADV*I; 16 is exact in fp8 and close to e^SHIFT
F32 = mybir.dt.float32
BF16 = mybir.dt.bfloat16
FP8 = mybir.dt.float8e4
AF = mybir.ActivationFunctionType
BF16NP = ml_dtypes.bfloat16
FP8NP = ml_dtypes.float8_e4m3fn

# chain placement: (partition base, alpha/output column)
CHHOME = [0, 64, 64, 0]
CHACOL = [0, 0, 1, 1]
GRP = 16          # positions per product group (4 quads -> 2 octs)
CHUNK = 64        # positions per DMA chunk (4 groups)


def split_multi_waits(nc, max_waits=1):
    """This walrus build accepts at most one sync-wait per instruction;
    move extra waits onto NOPs inserted just before, same engine."""
    for fn in nc.m.functions:
        for bb in fn.blocks:
            newl = []
            for ins in bb.instructions:
                si = ins.sync_info
                if si is not None and si.on_wait and len(si.on_wait) > max_waits:
                    waits = list(si.on_wait)
                    keep = waits[:max_waits]
                    extra = waits[max_waits:]
                    for i in range(0, len(extra), max_waits):
                        nop = mybir.InstNoOp(
                            name=nc.get_next_instruction_name(),
                            ins=[],
                            outs=[],
                            sync_info=mybir.SyncInfo(
                                on_wait=extra[i : i + max_waits], on_update=[]
                            ),
                        )
                        nop.engine = ins.engine
                        newl.append(nop)
                    si.on_wait = keep
                newl.append(ins)
            bb.instructions[:] = newl


def _chunk_layout(caps):
    """Static per-chunk layout shared by program and host packing.

    Returns a list of chunk dicts:
      k, lo (global position), npos, per-half: active chain list,
      region offset into that half's packed host array, region cols.
    """
    # graduated chunk sizes: small first chunks so the PE starts early
    los = [0, 16, 64]
    while los[-1] < max(caps):
        los.append(los[-1] + CHUNK)
    los = [x for x in los if x < max(caps)]
    chunks = []
    off = {0: 0, 64: 0}
    for k, lo in enumerate(los):
        nxt = los[k + 1] if k + 1 < len(los) else max(caps)
        npos = nxt - lo
        halves = {}
        for h in (0, 64):
            act = [c for c in range(4) if CHHOME[c] == h and caps[c] > lo]
            # all active chains cover the full chunk except possibly the
            # last positions; npos per half:
            nph = 0
            if act:
                nph = min(npos, max(caps[c] for c in act) - lo)
            halves[h] = dict(act=act, off=off[h], npos=nph)
            off[h] += nph * 64 * len(act)
        chunks.append(dict(k=k, lo=lo, halves=halves))
    return chunks, off[0], off[64]


def build_nc(caps):
    """caps: tuple of 4 per-chain capacities (multiples of GRP)."""
    chunks, totA, totB = _chunk_layout(caps)
    p8scale = float(np.exp(-8.0 * SHIFT))

    nc = bass.Bass()
    emA = nc.dram_tensor("emA", [64, totA], FP8, kind="ExternalInput")
    emB = nc.dram_tensor("emB", [64, totB], FP8, kind="ExternalInput")
    a0_d = nc.dram_tensor("a0", [128, 2], BF16, kind="ExternalInput")
    out_d = nc.dram_tensor("out", [128, 2], F32, kind="ExternalOutput")

    def em_ap(h, offset, ap):
        t = (emA if h == 0 else emB)[:, :].tensor
        return bass.AP(tensor=t, offset=offset, ap=ap)

    with tile.TileContext(nc) as tc:
        with (
            tc.tile_pool(name="expp", bufs=3) as expp,
            tc.tile_pool(name="p2sb", bufs=2) as p2sbp,
            tc.tile_pool(name="p4sb", bufs=2) as p4sbp,
            tc.tile_pool(name="p8sb", bufs=3) as p8sbp,
            tc.tile_pool(name="alpha", bufs=4) as alphap,
            tc.tile_pool(name="small", bufs=1) as small,
            tc.tile_pool(name="pp2", bufs=2, space="PSUM") as pp2p,
            tc.tile_pool(name="pp4", bufs=2, space="PSUM") as pp4p,
            tc.tile_pool(name="pp8", bufs=2, space="PSUM") as pp8p,
        ):
            # ---------------- init: alpha0 (host-prepared, exp domain)
            # alpha layout [128, 2]: chain c at (CHHOME[c], CHACOL[c])
            alpha = small.tile([128, 2], BF16)
            nc.sync.dma_start(out=alpha[:, :], in_=a0_d[:, :])
            # final alphas are snapshotted here as each chain finishes
            # (later batched scan casts clobber finished chains' columns)
            osb = small.tile([128, 2], F32)

            # ---------------- main pipeline over chunks
            for ch in chunks:
                lo = ch["lo"]
                hv = ch["halves"]
                ncols = {h: hv[h]["npos"] * 64 * len(hv[h]["act"]) for h in (0, 64)}
                et = expp.tile([128, max(ncols[0], ncols[64])], FP8, tag="exp")
                # issue bulk DMAs from the otherwise-idle gpsimd queue so
                # the sync queue's serial dma_start cost is off the ramp
                for h in (0, 64):
                    if ncols[h]:
                        tot = totA if h == 0 else totB
                        nc.gpsimd.dma_start(
                            out=et[h : h + 64, 0 : ncols[h]],
                            in_=em_ap(
                                h, hv[h]["off"], [[tot, 64], [1, ncols[h]]]
                            ),
                        )

                def esl(c, p):
                    # position p (global), chain c: slice of et
                    h = CHHOME[c]
                    a = hv[h]["act"]
                    off = (p - lo) * 64 * len(a) + 64 * a.index(c)
                    return et[h : h + 64, off : off + 64]

                # groups of GRP=16 positions
                glo = lo
                while glo < lo + max(
                    (hv[h]["npos"] for h in (0, 64) if hv[h]["act"]), default=0
                ):
                    gact = [c for c in range(4) if caps[c] > glo]
                    h0l = [c for c in gact if CHHOME[c] == 0]
                    h1l = [c for c in gact if CHHOME[c] == 64]
                    nh = max(len(h0l), len(h1l))
                    # emission order alternates partition halves so each
                    # quadrant's LDWEIGHTS prefetches during the other
                    # half's matmul
                    ordc = []
                    for i in range(nh):
                        if i < len(h0l):
                            ordc.append(h0l[i])
                        if i < len(h1l):
                            ordc.append(h1l[i])

                    def cix(c):
                        h = CHHOME[c]
                        return (h0l if h == 0 else h1l).index(c)

                    # P2 level: 8 pair products per chain (4 quads x 2)
                    pp2 = pp2p.tile([128, 1024], F32, tag="pp2")
                    for q in range(4):
                        p0 = glo + 4 * q
                        for half in range(2):
                            for c in ordc:
                                h = CHHOME[c]
                                cb = cix(c) * 512 + q * 128 + 64 * half
                                lhsT = esl(c, p0 + 1 + half)
                                rhs = esl(c, p0 + 3 * half)
                                nc.tensor.matmul(
                                    out=pp2[h : h + 64, cb : cb + 64],
                                    lhsT=lhsT,
                                    rhs=rhs,
                                    start=True,
                                    stop=True,
                                    tile_position=(h, h),
                                )
                    p2sb = p2sbp.tile([128, 1024], BF16, tag="p2sb")
                    uc = 512 * nh
                    nc.scalar.activation(
                        out=p2sb[:, 0 : uc // 2],
                        in_=pp2[:, 0 : uc // 2],
                        func=AF.Copy,
                    )
                    nc.vector.tensor_copy(
                        out=p2sb[:, uc // 2 : uc], in_=pp2[:, uc // 2 : uc]
                    )

                    # P4 level: 4 per chain; even quads operand-swapped so
                    # their P4 comes out transposed-stored
                    pp4 = pp4p.tile([128, 512], F32, tag="pp4")
                    for q in range(4):
                        for c in ordc:
                            h = CHHOME[c]
                            cb = cix(c) * 512 + q * 128
                            ob = cix(c) * 256 + q * 64
                            a_sl = p2sb[h : h + 64, cb : cb + 64]
                            b_sl = p2sb[h : h + 64, cb + 64 : cb + 128]
                            lhsT, rhs = (b_sl, a_sl) if q % 2 == 0 else (a_sl, b_sl)
                            nc.tensor.matmul(
                                out=pp4[h : h + 64, ob : ob + 64],
                                lhsT=lhsT,
                                rhs=rhs,
                                start=True,
                                stop=True,
                                tile_position=(h, h),
                            )
                    p4sb = p4sbp.tile([128, 512], BF16, tag="p4sb")
                    uc = 256 * nh
                    nc.scalar.activation(
                        out=p4sb[:, 0 : uc // 2], in_=pp4[:, 0 : uc // 2], func=AF.Copy
                    )
                    nc.vector.tensor_copy(
                        out=p4sb[:, uc // 2 : uc], in_=pp4[:, uc // 2 : uc]
                    )

                    # P8 level (2 per chain) + scan outputs share one tile
                    pp8 = pp8p.tile([128, 264], F32, tag="pp8")
                    for o in range(2):
                        for c in ordc:
                            h = CHHOME[c]
                            ci = cix(c)
                            ob4 = ci * 256 + o * 128
                            nc.tensor.matmul(
                                out=pp8[h : h + 64, ci * 128 + o * 64 :][:, 0:64],
                                lhsT=p4sb[h : h + 64, ob4 : ob4 + 64],
                                rhs=p4sb[h : h + 64, ob4 + 64 : ob4 + 128],
                                start=True,
                                stop=True,
                                tile_position=(h, h),
                            )
                    p8sb = p8sbp.tile([128, 256], BF16, tag="p8sb")
                    uc = 128 * nh
                    # the whole fp8 shift is paid once here: /e^{8*SHIFT}
                    nc.scalar.activation(
                        out=p8sb[:, 0 : uc // 2],
                        in_=pp8[:, 0 : uc // 2],
                        func=AF.Copy,
                        scale=p8scale,
                    )
                    nc.vector.tensor_scalar_mul(
                        out=p8sb[:, uc // 2 : uc],
                        in0=pp8[:, uc // 2 : uc],
                        scalar1=p8scale,
                    )

                    # scan: apply the two P8s in order; one batched
                    # [128,2] cast per oct covers all active chains
                    for o in range(2):
                        sc = 256 + 2 * o
                        for c in ordc:
                            h = CHHOME[c]
                            ci = cix(c)
                            nc.tensor.matmul(
                                out=pp8[h : h + 64, sc + CHACOL[c] :][:, 0:1],
                                lhsT=p8sb[h : h + 64, ci * 128 + o * 64 :][:, 0:64],
                                rhs=alpha[h : h + 64, CHACOL[c] : CHACOL[c] + 1],
                                start=True,
                                stop=True,
                                tile_position=(h, h),
                            )
                        at = alphap.tile([128, 2], BF16, tag="alpha")
                        nc.vector.tensor_copy(
                            out=at[:, :], in_=pp8[:, sc : sc + 2]
                        )
                        alpha = at
                    for c in gact:
                        if caps[c] == glo + GRP:
                            h = CHHOME[c]
                            nc.vector.tensor_copy(
                                out=osb[h : h + 64, CHACOL[c] : CHACOL[c] + 1],
                                in_=alpha[h : h + 64, CHACOL[c] : CHACOL[c] + 1],
                            )
                    glo += GRP

            # ---------------- finale: ship final alphas (fp32) to host
            nc.sync.dma_start(out=out_d[:, :], in_=osb[:, :])

    split_multi_waits(nc)
    return nc


_NC_CACHE = {}


def _get_nc(caps):
    if caps not in _NC_CACHE:
        _NC_CACHE[caps] = build_nc(caps)
    return _NC_CACHE[caps]


def prepare_inputs(emits, targets, mask):
    """Host-side prep: seq assignment, capacities, packed per-core arrays."""
    emits = np.asarray(emits, dtype=np.float32)
    maskb = np.asarray(mask).astype(bool)
    U = maskb[:, 1:].sum(axis=1).astype(np.int64)  # scan steps per seq

    # rank i (by U desc) -> core i%8, chain i//8
    order = np.argsort(-U, kind="stable")
    seq_of = np.empty((NCORES, 4), dtype=np.int64)
    for i, b in enumerate(order):
        seq_of[i % NCORES, i // NCORES] = b
    caps = tuple(
        int(-(-max(int(U[seq_of[j, c]]) for j in range(NCORES)) // GRP) * GRP)
        for c in range(4)
    )

    E4 = emits.reshape(B, S, L, L)
    # fp8 matrices: exp(E - C0 + SHIFT); the P2 cast divides by e^{2*SHIFT}
    X8 = np.exp(E4 - (C0 - SHIFT)).astype(FP8NP)                # [B,S,L,L]
    a0f = np.exp(E4[:, 0, 0, :] - C0).astype(BF16NP)            # [B, L]
    # pads are PADV*I (PADV exact in fp8, ~e^SHIFT so pads never underflow
    # bf16 alpha); the host adds (SHIFT - log PADV) per pad to logZ
    iden = (PADV * np.eye(L, dtype=np.float32)).astype(FP8NP)
    chunks, totA, totB = _chunk_layout(caps)

    in_maps = []
    for j in range(NCORES):
        emAa = np.empty((64, totA), dtype=FP8NP)
        emBa = np.empty((64, totB), dtype=FP8NP)
        a0 = np.zeros((128, 2), dtype=BF16NP)
        for c in range(4):
            b = seq_of[j, c]
            u = int(U[b])
            h = CHHOME[c]
            # chain matrices by position: steps 1..u, identity pad to cap
            G = np.empty((caps[c], L, L), dtype=FP8NP)
            G[:u] = X8[b, 1 : u + 1]
            G[u:] = iden
            G[0::2] = np.ascontiguousarray(G[0::2].swapaxes(-1, -2))
            arr = emAa if h == 0 else emBa
            for ch in chunks:
                hb = ch["halves"][h]
                act = hb["act"]
                if c not in act:
                    continue
                lo = ch["lo"]
                npos = min(hb["npos"], caps[c] - lo)
                view = arr[:, hb["off"] : hb["off"] + hb["npos"] * 64 * len(act)]
                view = view.reshape(64, hb["npos"], len(act), 64)
                view[:, 0:npos, act.index(c), :] = G[lo : lo + npos].transpose(
                    1, 0, 2
                )
            a0[h : h + 64, CHACOL[c]] = a0f[b]
        in_maps.append({"emA": emAa, "emB": emBa, "a0": a0})
    return in_maps, maskb, caps, seq_of, U


def host_score(emits, targets, maskb):
    tg = np.asarray(targets).astype(np.int64)
    idx = tg[:, :-1] * L + tg[:, 1:]                 # [B, S]
    em = np.asarray(emits, dtype=np.float64).reshape(B, S, L * L)
    gold = np.take_along_axis(em, idx[:, :, None], axis=-1)[..., 0]
    return float(np.where(maskb, gold, 0.0).sum())


def assemble_loss(results, maskb, score, seq_of, U, caps):
    logZ = 0.0
    for j in range(NCORES):
        o = np.asarray(results[j]["out"], dtype=np.float64)
        for c in range(4):
            b = seq_of[j, c]
            h = CHHOME[c]
            s = o[h : h + 64, CHACOL[c]].sum()
            # each PADV*I pad step nets alpha *= PADV/e^SHIFT through the
            # P2 cast; compensate exactly
            npad = caps[c] - int(U[b])
            logZ += (
                np.log(s)
                + C0 * (int(U[b]) + 1)
                + (SHIFT - np.log(PADV)) * npad
            )
    total_token = float(maskb.sum())
    return np.float32((logZ - score) / total_token)


def kernel(emits, targets, mask, _trace=False):
    in_maps, maskb, caps, seq_of, U = prepare_inputs(emits, targets, mask)
    score = host_score(emits, targets, maskb)
    nc = _get_nc(caps)
    res = run_bass_kernel_spmd(nc, in_maps, core_ids=list(range(NCORES)), trace=_trace)
    loss = assemble_loss(res.results, maskb, score, seq_of, U, caps)
    if _trace:
        return loss, res
    return loss
